# revision 79
# baseline (speedup 1.0000x reference)
"""Trainium2 Bass kernel for involution-style aggregation (SAN Aggregation).

Per batch element b (one per NeuronCore, pure data parallel over B=8):
    out[c, p] = sum_{idx in 0..8} xshift_idx[c, p] * w[c % 16, idx, p]
with (i, j) = (idx // 3, idx % 3), zero padding 1, K=3, stride 1, p = oh*64+ow.

Design (fp16 compute, ~57.0us/core simulated, scale-relative error ~1.2e-3):
- Channels on partitions (2 blocks of 128), spatial flattened on free dim.
- The host pre-builds three column-shift versions of x (dj = -1, 0, +1 with
  zero fill) in fp16 ("xv"), so every tap reduces to a row-shifted window of
  one version: a contiguous, even-offset 2D access pattern that keeps every
  DVE tensor_tensor in the 2x 16-bit perf mode. Row clipping handles the
  vertical shifts (taps cover 63 or 64 full rows); column zero padding lives
  inside the shift versions.
- Weight replication across the 8 channel groups: the host lays w out as
  [128, 9*512] ("w", partition s*16 + wc holding spatial chunk s, taps in
  TAP_ORDER column blocks). Seven taps are broadcast on-chip with k=128
  selector matmuls (exact for a 0/1 selector) into PSUM [128, 1024] tiles
  and copied to fp16 wtap tiles by the Scalar engine. The last two taps'
  replicated weights come directly from the host ("wlate"), loaded
  mid-kernel when the DMA pipe is idle, shortening ACT's serial queue.
- Tap-sum: block 0 cols 0:2048 accumulate on the TensorEngine as identity
  matmuls into two persistent PSUM quarters (accumulation group opened by
  the first tap, closed by the last, drained by ACT). The remaining 6144
  columns accumulate via three independent in-place add half-chains on DVE.
  The last tap's multiplies are split into chain-aligned halves and the
  output DMAs are ordered by readiness, shortening the tail critical path
  (last product -> final adds -> final store).
- The 18 multiplies are split between DVE (slots in DVE_MULT_SLOTS) and
  GPSIMD so both engines finish together (DVE fp16 TT ~2.2us, GPSIMD ~3.4us
  per [128, 4096] op; all engine assignments tuned against the CoreSim
  cost model).
- Output is stored fp16 and upcast to f32 on the host.
- _legalize_sync_waits rewrites the scheduled IR so no instruction carries
  more than one sync wait (walrus codegen limit in this toolchain).
"""

import sys

for _p in (
    "/root/.axon_site",
    "/root/.axon_site/_ro/trn_rl_repo",
    "/root/.axon_site/_ro/pypackages",
):
    if _p not in sys.path:
        sys.path.append(_p)

from contextlib import ExitStack

import numpy as np

import concourse.bass as bass
import concourse.tile as tile
from concourse import mybir
from concourse.bass_utils import run_bass_kernel_spmd

B, C, H, W = 8, 256, 64, 64
WC, K2 = 16, 9
OH, OW = 64, 64
P = OH * OW
N_CORES = 8
F32 = mybir.dt.float32
F16 = mybir.dt.float16

# Tap order: center first so the first op fully initializes acc; then the
# other column-centered taps (j=1, same x version as center, already loaded),
# then j=0 taps, then j=2 taps — matching the x-version DMA order (1, 0, 2).
TAP_ORDER = [4, 1, 7, 0, 3, 6, 2, 5, 8]
# Both blocks' add chains run on DVE (2.17us/op, two independent chains
# interleave so chain latency never binds); GPSIMD is a pure multiply
# producer. A few multiplies stay on DVE to balance engine finish times;
# DVE_MULT_SLOTS picks which of the 16 non-center multiplies those are.
DVE_MULT_SLOTS = frozenset((1, 3, 6, 9, 12, 16))
# Engine per add chain: "v" = DVE, "g" = GPSIMD
CHAIN_ENGINES = {"b0h1": "v", "b1h0": "v", "b1h1": "v"}


def _legalize_sync_waits(nc, max_waits: int = 1) -> int:
    """Walrus codegen rejects instructions with >1 sync wait. Hoist excess
    waits onto same-engine drain carriers inserted just before the
    over-subscribed instruction (per-engine program order preserved)."""
    n_moved = 0
    counter = [0]
    for func in nc.m.functions:
        for bb in func.blocks:
            insts = list(bb.instructions)
            out = []
            changed = False
            for inst in insts:
                si = inst.sync_info
                waits = list(si.on_wait) if (si and si.on_wait) else []
                if len(waits) > max_waits:
                    extra, keep = waits[:-max_waits], waits[-max_waits:]
                    for w in extra:
                        counter[0] += 1
                        # NoOp, not Drain: carries the wait without flushing
                        # the engine pipeline
                        carrier = mybir.InstNoOp(
                            name=f"{inst.name}_wsplit{counter[0]}", ins=[], outs=[]
                        )
                        carrier.engine = inst.engine
                        carrier.sync_info = mybir.SyncInfo(on_wait=[w], on_update=[])
                        out.append(carrier)
                        n_moved += 1
                    si.on_wait = keep
                    changed = True
                out.append(inst)
            if changed:
                try:
                    bb.instructions = out
                except Exception:
                    cur = bb.instructions
                    cur[:] = out
    return n_moved


def _selector8() -> np.ndarray:
    """[128, 8*128] bank of selectors (bf16-exact 0/1 values). Selector s has
    sel[k, c] = 1 iff k == s*16 + c % 16: a k=128 matmul against the
    (s*16+wc)-partitioned weight buffer broadcasts spatial chunk s's weights
    to all 128 output channels."""
    sel = np.zeros((128, 8 * 128), dtype=np.float32)
    for s in range(8):
        for c in range(128):
            sel[s * WC + c % WC, s * 128 + c] = 1.0
    return sel


def _build(legalize: bool = True):
    nc = bass.Bass()
    # xv: [3, C, P] column-shift versions of x (dj = -1, 0, +1), fp16.
    xv = nc.declare_dram_parameter("xv", [3, C, P], F16, isOutput=False)
    # w pre-laid-out on host: [128, K2*512], partition (s*16 + wc) holds
    # w[wc, TAP_ORDER[ord], s*512:(s+1)*512] at columns ord*512.
    w = nc.declare_dram_parameter("w", [128, K2 * 512], F16, isOutput=False)
    # Last two taps' weights pre-replicated on host ([2, 128, P]): their
    # wtaps skip the PE->ACT broadcast and load mid-kernel when the DMA pipe
    # is otherwise idle, shortening ACT's serial wtap queue.
    wlate = nc.declare_dram_parameter("wlate", [2, 128, P], F16, isOutput=False)
    out = nc.declare_dram_parameter("out", [C, P], F16, isOutput=True)
    sel_np = np.concatenate(
        [_selector8(), np.eye(128, dtype=np.float32)], axis=1
    ).astype(np.float16)
    sel_d = nc.inline_tensor(sel_np, name="sel")

    with tile.TileContext(nc) as tc:
        with ExitStack() as ctx:
            selp = ctx.enter_context(tc.tile_pool(name="sel", bufs=1))
            xp = ctx.enter_context(tc.tile_pool(name="xb", bufs=1))
            wsp = ctx.enter_context(tc.tile_pool(name="wsb", bufs=1))
            wt = ctx.enter_context(tc.tile_pool(name="wt", bufs=3))
            ps = ctx.enter_context(tc.tile_pool(name="ps", bufs=2, space="PSUM"))
            pa = ctx.enter_context(tc.tile_pool(name="pa", bufs=1, space="PSUM"))
            tp = ctx.enter_context(tc.tile_pool(name="tmp", bufs=5))
            ap = ctx.enter_context(tc.tile_pool(name="acc", bufs=1))

            sel_t = selp.tile([128, 9 * 128], F16)
            nc.sync.dma_start(sel_t[:], sel_d[:])
            ident = sel_t[:, 8 * 128 : 9 * 128]
            # Persistent PSUM accumulators for block 0, cols 0:2048 — the
            # tap-sum for these columns runs on the TensorEngine as identity
            # matmuls with PSUM accumulation (start on first tap, stop on
            # last), freeing DVE/GPSIMD adds.
            ps_acc = [
                pa.tile([128, 1024], F32, tag=f"pacc{q}", name=f"pacc{q}")
                for q in range(2)
            ]
            # Warm the ACT engine's function table before the first real
            # PSUM->SBUF copy (the first Activation otherwise pays ~1.3us).
            warm = selp.tile([128, 2], F16, name="warm")
            nc.scalar.activation(
                warm[:], sel_t[:, 0:2], mybir.ActivationFunctionType.Copy
            )

            # Weight buffer: partition (s*16 + wc) holds w[wc, idx, s*512 +
            # 0:512] for each tap, in TAP_ORDER column blocks (ord*512).
            # The per-tap weight loads and per-(block, version) x loads are
            # interleaved on the single serialized DMA pipe so the
            # PE->ACT->DVE pipeline and both MAC engines ramp as early as
            # possible.
            wsb = wsp.tile([128, K2 * 512], F16)
            xt = []
            accs = []
            for blk in range(2):
                t = xp.tile([128, 3, P], F16, tag=f"xt{blk}", name=f"xt{blk}")
                xt.append(t)
                accs.append(ap.tile([128, P], F16, tag=f"acc{blk}", name=f"acc{blk}"))

            def load_w(lo, hi):
                nc.sync.dma_start(wsb[:, lo * 512 : hi * 512], w[:, lo * 512 : hi * 512])

            def load_x(blk, v):
                nc.sync.dma_start(xt[blk][:, v], xv[v, blk * 128 : (blk + 1) * 128])

            load_w(0, 1)
            load_x(0, 1)
            load_w(1, 3)
            load_x(1, 1)
            load_w(3, 6)
            load_x(0, 0)
            load_x(1, 0)
            load_w(6, 7)
            load_x(0, 2)
            load_x(1, 2)
            wlate_t = []
            for k in range(2):
                t = wt.tile([128, P], F16, tag=f"wlate{k}", name=f"wlate{k}")
                nc.sync.dma_start(t[:], wlate[k])
                wlate_t.append(t)

            n_mults = 0
            # per-chain state: (prev tmp AP, prev tap row offset) until the
            # first binary add writes acc
            chain_first = {"b0h1": None, "b1h0": None, "b1h1": None}
            for ord_, idx in enumerate(TAP_ORDER):
                # --- replicate w[:, idx, :] across the 8 channel groups ---
                if ord_ >= K2 - 2:
                    wtap = wlate_t[ord_ - (K2 - 2)]
                else:
                    wtap = wt.tile([128, P], F16)
                    for quarter in range(4):
                        pst = ps.tile([128, 1024], F32)
                        for ch in range(2):
                            s = quarter * 2 + ch
                            nc.tensor.matmul(
                                pst[:, ch * 512 : (ch + 1) * 512],
                                sel_t[:, s * 128 : (s + 1) * 128],
                                wsb[:, ord_ * 512 : (ord_ + 1) * 512],
                                start=True,
                                stop=True,
                            )
                        nc.scalar.activation(
                            wtap[:, quarter * 1024 : (quarter + 1) * 1024],
                            pst[:],
                            mybir.ActivationFunctionType.Copy,
                        )

                # --- tap geometry: row-shifted window of version v = j ---
                i, j = divmod(idx, 3)
                di = i - 1
                r0, rows = max(0, -di), OH - abs(di)
                n = rows * OW
                o0 = r0 * OW  # acc-plane offset of this tap's contribution
                tmps = []
                for blk in range(2):
                    xs = xt[blk][:, j, (r0 + di) * OW : (r0 + di) * OW + n]
                    wv = wtap[:, o0 : o0 + n]
                    mul_eng = nc.vector if n_mults in DVE_MULT_SLOTS else nc.gpsimd
                    n_mults += 1
                    tmp = tp.tile([128, P], F16)
                    if ord_ == K2 - 1:
                        # last tap: multiply in halves, the chain-consumed
                        # half first, so the tail adds unblock sooner
                        first = 2048 if blk == 0 else 0
                        for lo2 in (first, 2048 - first):
                            l2, h2 = max(lo2, 0), min(lo2 + 2048, n)
                            mul_eng.tensor_mul(
                                tmp[:, l2:h2], xs[:, l2:h2], wv[:, l2:h2]
                            )
                    else:
                        mul_eng.tensor_mul(tmp[:, 0:n], xs, wv)
                    tmps.append(tmp)

                # --- block 0, cols 0:2048: PE identity-matmul accumulation
                # into the two persistent PSUM quarters, in bank-aligned
                # 512-col chunks clipped to the tap's valid range (row-clipped
                # taps miss cols [0:64) or [4032:4096) of the plane; within
                # 0:2048 only the [0:64) clip matters). The center tap
                # (ord 0) covers everything and opens the accumulation group;
                # the last tap covers [0:4032) and closes it.
                for q in range(2):
                    qlo = q * 1024
                    for bank in range(2):
                        blo, bhi = qlo + bank * 512, qlo + (bank + 1) * 512
                        lo, hi = max(blo, o0), min(bhi, o0 + n)
                        if lo >= hi:
                            continue
                        nc.tensor.matmul(
                            ps_acc[q][:, lo - qlo : hi - qlo],
                            ident[:],
                            tmps[0][:, lo - o0 : hi - o0],
                            start=(ord_ == 0),
                            stop=(ord_ == K2 - 1),
                            skip_group_check=True,
                        )

                # --- block 0 cols 2048:4096 + block 1 (two half-chains):
                # independent add chains, engine per CHAIN_ENGINES
                for key, blk, alo, ahi in (
                    ("b0h1", 0, 2048, 4096),
                    ("b1h0", 1, 0, 2048),
                    ("b1h1", 1, 2048, 4096),
                ):
                    eng = nc.vector if CHAIN_ENGINES.get(key, "v") == "v" else nc.gpsimd
                    lo, hi = max(alo, o0), min(ahi, o0 + n)
                    av = accs[blk][:, lo:hi]
                    tv = tmps[blk][:, lo - o0 : hi - o0]
                    if chain_first[key] is None:
                        # center tap: stash; it covers the whole plane
                        chain_first[key] = tmps[blk]
                    elif chain_first[key] != "done":
                        ptmp = chain_first[key]
                        eng.tensor_add(av, ptmp[:, lo:hi], tv)
                        if lo > alo:
                            eng.tensor_copy(accs[blk][:, alo:lo], ptmp[:, alo:lo])
                        if hi < ahi:
                            eng.tensor_copy(accs[blk][:, hi:ahi], ptmp[:, hi:ahi])
                        chain_first[key] = "done"
                    else:
                        eng.tensor_add(av, av, tv)

            # drain the PSUM quarters into acc block 0 (f32 -> f16)
            for q in range(2):
                nc.scalar.activation(
                    accs[0][:, q * 1024 : (q + 1) * 1024],
                    ps_acc[q][:],
                    mybir.ActivationFunctionType.Copy,
                )

            # chain-produced regions flush as their chains complete; the
            # PSUM-drained half of block 0 goes last
            nc.sync.dma_start(out[0:128, 2048:4096], accs[0][:, 2048:4096])
            nc.sync.dma_start(out[0:128, 0:2048], accs[0][:, 0:2048])
            nc.sync.dma_start(out[128:256, 0:2048], accs[1][:, 0:2048])
            nc.sync.dma_start(out[128:256, 2048:4096], accs[1][:, 2048:4096])

    if legalize:
        _legalize_sync_waits(nc)
    return nc


_NC_CACHE = {}


def get_nc(legalize: bool = True):
    key = "nc_legal" if legalize else "nc_raw"
    if key not in _NC_CACHE:
        _NC_CACHE[key] = _build(legalize)
    return _NC_CACHE[key]


def _make_xv(x: np.ndarray) -> np.ndarray:
    """[3, C, P] fp16 column-shift versions of one batch element's x
    ([C, H, W] f32): version v reads x[., ., w + (v-1)] with zero fill."""
    xb = x.astype(np.float16)
    xvs = np.zeros((3, C, H, W), dtype=np.float16)
    xvs[0, :, :, 1:] = xb[:, :, :-1]  # v=0: dj=-1 -> x[., w-1]
    xvs[1] = xb
    xvs[2, :, :, :-1] = xb[:, :, 1:]  # v=2: dj=+1 -> x[., w+1]
    return xvs.reshape(3, C, P)


def _make_wsb(wb: np.ndarray) -> np.ndarray:
    """[128, K2*512] weight layout for one batch element ([WC, K2, P] fp16):
    partition (s*16 + wc) holds w[wc, TAP_ORDER[ord], s*512:(s+1)*512] at
    column block ord*512."""
    wt = wb[:, TAP_ORDER, :].reshape(WC, K2, 8, 512)
    return np.ascontiguousarray(
        wt.transpose(2, 0, 1, 3).reshape(128, K2 * 512)
    )


def kernel(x: np.ndarray, weight: np.ndarray) -> np.ndarray:
    x = np.ascontiguousarray(np.asarray(x, dtype=np.float32))
    weight = np.ascontiguousarray(np.asarray(weight, dtype=np.float32))
    assert x.shape == (B, C, H, W), x.shape
    assert weight.shape == (B, WC, K2, P), weight.shape

    nc = get_nc()
    wb = weight.astype(np.float16)
    in_maps = [
        {
            "xv": _make_xv(x[i]),
            "w": _make_wsb(wb[i]),
            "wlate": np.ascontiguousarray(
                np.stack(
                    [np.tile(wb[i][:, t, :], (8, 1)) for t in TAP_ORDER[-2:]]
                )
            ),
        }
        for i in range(N_CORES)
    ]
    res = run_bass_kernel_spmd(nc, in_maps, list(range(N_CORES)))
    out = np.stack([res.results[i]["out"] for i in range(N_CORES)], axis=0)
    return out.reshape(B, C, H, W).astype(np.float32)
